# revision 9
# baseline (speedup 1.0000x reference)
"""Cumulative-FFT Trainium2 kernel (v2: stationary-build scheme).

out[b,t,d,k,c] = pos_norm[t] * cumsum_t( x[b,t,d] * twiddles[t,k,c] )

Shapes (hardcoded): x (4,1024,512) bf16, twiddles (1024,32,2) bf16,
pos_norm (1024,) bf16  ->  out (4,1024,512,32,2) bf16.

Sharding: 8 cores = batch(4) x d_model-half(2). Each core computes a
(1024, 256*64) bf16 shard (32 MiB).

v1 (145.9us) built the per-block moving operand c[s, kc*256+d] =
x[s,d]*tw[s,kc] with a DVE tensor_tensor (16384 elems/partition @2x =
8.6us/block) and did the cumsum as utri^T @ c. DVE was the binding
engine: TT 8.6 + 4 PSUM evictions = 13.46us/block while the store DMA
floor is only 11.7us/block (32 MiB @ 358 GB/s).

v2 moves the twiddle multiply into the PE stationary: per block j and
frequency-pair kc, the stationary is A_kc[s,t] = utri[s,t]*tw[t0+s,kc]
(utri carries pos_norm), built by ONE DVE tensor_tensor over only
[128, 64*128] = 8192 elems/partition (4.33us/block, half of v1), and
the moving operand is the shared x block (no per-column build at all).
The carry (column sums of previous blocks) can no longer ride row 0 of
the moving operand, so it is added into PSUM by rank-1 matmuls:
psum[t,n] += posrow[t]*carry_flat[n], K=1, four of them packed onto
the four 32-row PE tile positions so they run concurrently (~0.5us of
PE per batch of 4 groups instead of 1.7us unpacked). carry_flat is a
[1,4096]-per-lane flattening of the [64,256] running carry, done by a
tiny HWDGE SBUF->SBUF DMA on the otherwise idle scalar queue.

Per-block steady-state budget (target 11.7us = DMA store floor):
  PE   : 64 (LDW+MM 256-col) + 16 K=1 carry MMs + delta ~= 10.1us
  DVE  : carry add 0.33 + A-build 4.33 + 6 evictions ~= 11.5us
  ACT  : 10 evictions ~= 10.1us
  DMA  : 4 MiB stores ~= 11.7us  <- binding
"""

import sys

sys.path.insert(0, "/opt/trn_rl_repo")

import ml_dtypes
import numpy as np

import concourse.bass as bass
import concourse.mybir as mybir
import concourse.tile as tile
from concourse import bacc
import concourse.bass_utils as _bu
from concourse.bass_utils import run_bass_kernel_spmd

B, T, D = 4, 1024, 512
KC = 64            # 32 freqs x (cos,sin), flattened innermost dims of out
DSH = D // 2       # d-slice per core
NKC = DSH * KC     # free elements per t per core (16384)
BLK = 128          # rows per t-block
NBLK = T // BLK    # 8

BF16 = mybir.dt.bfloat16
F32 = mybir.dt.float32

# eviction split: DVE evicts the tail groups (its queue drains the next
# block's A-build first), ACT the head groups
_DVE_GROUPS = (10, 11, 12, 13, 14, 15)
_DVE_GROUPS_LAST = (1, 3, 5, 7, 9, 11, 13, 15)
# column-chunk stores, keyed by the group index after whose eviction op
# they are emitted. Queue (FIFO) order must match expected eviction
# completion: ACT walks 0..9 (~1.0us each) while DVE does 10..15 after
# its A-build (~4.7us in, ~1.13us each), so (10,3) completes before
# (5,5) and must be queued ahead of it.
_STORE_AFTER = {4: [(0, 5)], 12: [(10, 3), (5, 5)], 15: [(13, 3)]}
_STORE_AFTER_LAST = {
    2: [(0, 3)], 5: [(3, 3)], 7: [(6, 2)], 9: [(8, 2)],
    11: [(10, 2)], 13: [(12, 2)], 15: [(14, 2)],
}

LAST_RESULTS = None  # set by kernel(); test.py reads exec_time_ns from here


def _build_utri(pos_norm: np.ndarray) -> np.ndarray:
    """Stationary triangular masks for all blocks, packed (128, NBLK*128)."""
    pos = np.asarray(pos_norm).astype(np.float32)
    utri = np.zeros((128, NBLK * 128), np.float32)
    s = np.arange(128)[:, None]
    t = np.arange(128)[None, :]
    for k in range(NBLK):
        t0 = k * BLK
        utri[:, 128 * k : 128 * (k + 1)] = (s <= t) * pos[t0 : t0 + 128][None, :]
    return utri.astype(ml_dtypes.bfloat16)


def _build_posr(pos_norm: np.ndarray) -> np.ndarray:
    """pos rows for the K=1 carry matmuls at partitions 0/32/64/96."""
    pos = np.asarray(pos_norm).astype(np.float32)
    posr = np.zeros((128, NBLK * 128), np.float32)
    for g in range(4):
        posr[32 * g, :] = pos
    return posr.astype(ml_dtypes.bfloat16)


def _build_program() -> bass.Bass:
    nc = bacc.Bacc("TRN2", target_bir_lowering=False, debug=False)
    x_d = nc.dram_tensor("x_shard", [T, DSH], BF16, kind="ExternalInput").ap()
    tw_d = nc.dram_tensor("tw", [T, KC], BF16, kind="ExternalInput").ap()
    utri_d = nc.dram_tensor("utri", [128, NBLK * 128], BF16, kind="ExternalInput").ap()
    posr_d = nc.dram_tensor("posr", [128, NBLK * 128], BF16, kind="ExternalInput").ap()
    twrep_d = nc.dram_tensor("twrep", [T, KC * 16], BF16, kind="ExternalInput").ap()
    out_d = nc.dram_tensor("out_shard", [T, NKC], BF16, kind="ExternalOutput").ap()

    with tile.TileContext(nc) as tc:
        with (
            tc.tile_pool(name="singles", bufs=1) as singles,
            tc.tile_pool(name="ap_", bufs=2) as ap_,
            tc.tile_pool(name="outp", bufs=2) as outp,
            tc.tile_pool(name="carryp", bufs=3) as carryp,
            tc.tile_pool(name="cfp", bufs=2) as cfp,
            tc.tile_pool(name="pmain", bufs=4, space="PSUM") as pmain,
        ):
            # bulk loads, all 128-partition (16-way SDMA striping).
            # sync queue: tw (delta MMs need it early), then x.
            tw_all = singles.tile([128, NBLK * KC], BF16)
            nc.sync.dma_start(
                out=tw_all.rearrange("p (j k) -> p j k", j=NBLK),
                in_=tw_d.rearrange("(j p) k -> p j k", p=128),
            )
            x_all = singles.tile([128, NBLK * DSH], BF16)
            x_all_v = x_all.rearrange("p (j d) -> p j d", j=NBLK)
            x_d_v = x_d.rearrange("(j p) d -> p j d", p=128)
            nc.sync.dma_start(out=x_all_v[:, 0:2], in_=x_d_v[:, 0:2])
            nc.sync.dma_start(out=x_all_v[:, 2:NBLK], in_=x_d_v[:, 2:NBLK])
            # scalar queue: rep block 0 (gates A-build 0), utri, posr, rest
            rep_all = singles.tile([128, NBLK * KC * 16], BF16)
            rep_v0 = rep_all.rearrange("p (j r) -> p j r", j=NBLK)
            twrep_v = twrep_d.rearrange("(j p) r -> p j r", p=128)
            nc.scalar.dma_start(out=rep_v0[:, 0:1], in_=twrep_v[:, 0:1])
            utri_sb = singles.tile([128, NBLK * 128], BF16)
            nc.scalar.dma_start(out=utri_sb[:, :], in_=utri_d[:, :])
            posr_sb = singles.tile([128, NBLK * 128], BF16)
            nc.scalar.dma_start(out=posr_sb[:, :], in_=posr_d[:, :])
            nc.scalar.dma_start(out=rep_v0[:, 1:NBLK], in_=twrep_v[:, 1:NBLK])

            def build_a(k, nchunks=1):
                # stationary build: a[s, kc*128+t] = utri[s, k*128+t]*tw[k*128+s, kc]
                # one DVE tensor_tensor in 2x mode: utri broadcast over kc
                # (outermost 0-stride), tw via the 16x-replicated tile so the
                # innermost dim has stride 1 (same AP shape as v1's c-build).
                rep16 = rep_all[:, k * KC * 16 : (k + 1) * KC * 16]
                ut = utri_sb[:, k * 128 : (k + 1) * 128]
                a_sb = ap_.tile([128, KC * 128], BF16)
                ut_v3 = ut.rearrange("p (b c) -> p b c", c=16).unsqueeze(1)
                rep_v3 = rep16.rearrange("p (a c) -> p a c", c=16).unsqueeze(2)
                kcn = KC // nchunks
                for ci in range(nchunks):
                    ka, kb = ci * kcn, (ci + 1) * kcn
                    a_v = a_sb[:, ka * 128 : kb * 128].rearrange(
                        "p (a b c) -> p a b c", b=8, c=16
                    )
                    nc.vector.tensor_mul(
                        a_v,
                        ut_v3.broadcast_to((128, kcn, 8, 16)),
                        rep_v3[:, ka:kb].broadcast_to((128, kcn, 8, 16)),
                    )
                return a_sb

            # software pipeline: block k's matmuls consume the A tile and the
            # carry_flat built during block k-1
            carry_prev = None
            cf_prev = None
            a_cur = build_a(0, nchunks=4)
            cf_cur = None
            for k in range(NBLK):
                if k + 1 < NBLK:
                    # carry for the next block: += tw_k^T @ x_k
                    delta = pmain.tile([KC, DSH], F32, tag="pg")
                    nc.tensor.matmul(
                        delta[:, :],
                        lhsT=tw_all[:, k * KC : (k + 1) * KC],
                        rhs=x_all[:, k * DSH : (k + 1) * DSH],
                        start=True, stop=True,
                    )
                    carry_new = carryp.tile([KC, DSH], BF16)
                    if carry_prev is None:
                        nc.vector.tensor_copy(carry_new[:, :], delta[:, :])
                    else:
                        nc.vector.tensor_add(
                            carry_new[:, :], carry_prev[:, :], delta[:, :]
                        )
                    carry_prev = carry_new
                    # flatten [64,256] -> per-lane [1,4096] rows at partitions
                    # 0/32/64/96. The delta matmul's tw columns are permuted on
                    # the host so lane g's 16 carry rows are contiguous
                    # [16g:16g+16] (a strided-partition source AP reads wrong
                    # data); SWDGE handles the partition->free fold.
                    cf_new = cfp.tile([128, 4 * 1024], BF16)
                    for g in range(4):
                        nc.gpsimd.dma_start(
                            out=cf_new[32 * g : 32 * g + 1, :],
                            in_=carry_new[16 * g : 16 * (g + 1), :],
                        )
                    cf_next = cf_new
                    a_next = build_a(k + 1)
                else:
                    a_next = None
                    cf_next = None

                og = outp.tile([128, NKC], BF16)
                dve_groups = _DVE_GROUPS_LAST if k == NBLK - 1 else _DVE_GROUPS
                store_after = _STORE_AFTER_LAST if k == NBLK - 1 else _STORE_AFTER
                n_ship = 0
                n_chunks = len([c for v in store_after.values() for c in v])
                for bb in range(4):           # batch of 4 groups
                    pgs = [
                        pmain.tile([128, 1024], F32, tag="pg", name=f"pg_{k}_{bb}_{g}")
                        for g in range(4)
                    ]
                    if cf_cur is not None:
                        # 8 K=1 rank-1 carry matmuls, 4-way row-packed:
                        # psum[t, n] += posr[t] * carry_flat[n]
                        for half in range(2):
                            for g in range(4):
                                nc.tensor.matmul(
                                    pgs[g][:, half * 512 : (half + 1) * 512],
                                    lhsT=posr_sb[
                                        32 * g : 32 * g + 1,
                                        k * 128 : (k + 1) * 128,
                                    ],
                                    rhs=cf_cur[
                                        32 * g : 32 * g + 1,
                                        bb * 1024 + half * 512
                                        : bb * 1024 + (half + 1) * 512,
                                    ],
                                    start=True, stop=False,
                                    tile_position=(32 * g, 0),
                                    skip_group_check=True,
                                )
                    for g in range(4):
                        gi = bb * 4 + g       # group index 0..15
                        pg = pgs[g]
                        for q in range(4):
                            kc = gi * 4 + q
                            nc.tensor.matmul(
                                pg[:, q * 256 : (q + 1) * 256],
                                lhsT=a_cur[:, kc * 128 : (kc + 1) * 128],
                                rhs=x_all[:, k * DSH : (k + 1) * DSH],
                                start=(cf_cur is None),
                                stop=True,
                                skip_group_check=True,
                            )
                        col = gi * 1024
                        if gi in dve_groups:
                            nc.vector.tensor_copy(
                                og[:, col : col + 1024], pg[:, :]
                            )
                        else:
                            nc.scalar.copy(
                                og[:, col : col + 1024], pg[:, :]
                            )
                        for (c0, cg) in store_after.get(gi, []):
                            n_ship += 1
                            a = c0 * 1024
                            bcol = (c0 + cg) * 1024
                            eng = (
                                nc.scalar
                                if k == NBLK - 1 and n_ship > n_chunks - 3
                                else nc.sync
                            )
                            eng.dma_start(
                                out=out_d[k * BLK : (k + 1) * BLK, a:bcol],
                                in_=og[:, a:bcol],
                            )
                a_cur = a_next
                cf_cur = cf_next
    nc.compile()
    return nc


def kernel(**inputs) -> np.ndarray:
    global LAST_RESULTS
    x = np.asarray(inputs["x"])                       # (4,1024,512) bf16
    tw = np.asarray(inputs["twiddles"])               # (1024,32,2) bf16
    pos = np.asarray(inputs["pos_norm"])              # (1024,) bf16

    tw2 = np.ascontiguousarray(tw.reshape(T, KC))
    twrep = np.ascontiguousarray(np.repeat(tw2, 16, axis=1))
    utri = _build_utri(pos)
    posr = _build_posr(pos)
    # delta-path tw with permuted columns: carry row r=16g+4b+c holds
    # kc=16b+4g+c, so lane g's 16 rows (batches b, chunks c) are contiguous
    r = np.arange(KC)
    kc_map = 16 * ((r % 16) // 4) + 4 * (r // 16) + (r % 4)
    tw2p = np.ascontiguousarray(tw2[:, kc_map])

    in_maps = []
    for core in range(8):
        b, dh = core // 2, core % 2
        xs = np.ascontiguousarray(x[b, :, dh * DSH : (dh + 1) * DSH])
        in_maps.append(
            {"x_shard": xs, "tw": tw2p, "utri": utri, "posr": posr, "twrep": twrep}
        )

    nc = _build_program()
    res = run_bass_kernel_spmd(nc, in_maps, core_ids=list(range(8)))
    LAST_RESULTS = res

    out = np.empty((B, T, D, KC // 2, 2), dtype=x.dtype)
    for core in range(8):
        b, dh = core // 2, core % 2
        o = np.asarray(res.results[core]["out_shard"])  # (T, NKC) kc-major
        o = o.reshape(T, KC, DSH).transpose(0, 2, 1)    # -> (T, DSH, KC)
        out[b, :, dh * DSH : (dh + 1) * DSH, :, :] = o.reshape(T, DSH, KC // 2, 2)
    return out


if __name__ == "__main__":
    rng = np.random.default_rng(0)
    demo = {
        "x": rng.standard_normal((B, T, D), np.float32).astype(ml_dtypes.bfloat16),
        "twiddles": rng.standard_normal((T, KC // 2, 2), np.float32).astype(
            ml_dtypes.bfloat16
        ),
        "pos_norm": (1.0 / np.sqrt(np.arange(1, T + 1, dtype=np.float32))).astype(
            ml_dtypes.bfloat16
        ),
    }
    print(kernel(**demo).shape)


# revision 11
# speedup vs baseline: 1.2193x; 1.2193x over previous
"""Cumulative-FFT Trainium2 kernel (v3).

out[b,t,d,k,c] = pos_norm[t] * cumsum_t( x[b,t,d] * twiddles[t,k,c] )

Shapes (hardcoded): x (4,1024,512) bf16, twiddles (1024,32,2) bf16,
pos_norm (1024,) bf16  ->  out (4,1024,512,32,2) bf16.

Sharding: 8 cores = batch(4) x d_model-half(2). Each core computes a
(1024, 256*64) bf16 shard (32 MiB) -- data-parallel over B, tensor-parallel
over D, nothing crosses cores.

Per-core algorithm (v1 lineage): cumsum along t as per-block triangular
matmuls on the PE; the moving operand c holds the bf16 contributions
c[s, kc*256+d] = x[s,d]*tw[s,kc] (one 2x-mode DVE tensor_tensor against a
16x-replicated tw tile, 8.6us/block); the carry (column sums of previous
blocks) is folded into c's row 0 by an accumulating SWDGE DMA, so
utri[s,t] = pos[t0+t]*(s<=t) finishes each block in one matmul pass.

v3 changes over the 145.9us v1:
 - The whole carry chain (7 delta matmuls tw_k^T @ x_k + DVE adds) runs
   up front, right after the loads: carries stop gating late blocks
   (v1 lost ~5us waiting for carry_7 after TT_7).
 - Eviction split alternates 3/4 DVE groups per block (v1 fixed 4),
   balancing DVE (TT 8.6 + casts) against ACT across block pairs:
   2-block totals DVE 25.1us / ACT 25.1us -> ~12.6us/block steady
   (v1: 13.46).
 - Stores alternate between the qSync and qScalar HW-DGE queues (4
   chunks of 4 groups per block), halving per-queue load.
 - Head loads reordered: x block 0 + rep block 0 + utri first, so TT_0
   starts ~3us earlier.

A failed v2 for the record: moving the tw multiply into the PE
stationary (A_kc = utri*tw, built on DVE at half the TT cost) requires
re-adding the carry via K=1 rank-1 matmuls; ANY partial-K matmul (K=1
or K=32, packed or not) permanently throttles the PE clock to 1.2 GHz
(HAM never un-throttles; measured 75us of gapless back-to-back MMs all
at the cold rate), and full-K carry matmuls cost their column count
(+6.8us/block). Hard constraint: keep every matmul K=128.

Hard-won trace facts (v1, still binding):
 - HWDGE stripes a DMA across 16 SDMA engines only when the partition
   count divides by 16; all bulk DMAs here are 128-partition.
 - DVE TENSOR_TENSOR bf16 is capped at 2x mode ((58+FD/2)/0.96GHz);
   PSUM-source evictions are capped at 1x on both DVE ((120+FD)/0.96)
   and ACT ((172+FD)/1.2).
 - PSUM is 8 banks: pmain bufs=4 x 2 banks; the delta matmuls share the
   rotation via tag so PSUM never exceeds 8 banks.
 - Store floor: 32 MiB @ ~358 GB/s HBM-per-core = 11.7us/block.
"""

import sys

sys.path.insert(0, "/opt/trn_rl_repo")

import ml_dtypes
import numpy as np

import concourse.bass as bass
import concourse.mybir as mybir
import concourse.tile as tile
from concourse import bacc
import concourse.bass_utils as _bu
from concourse.bass_utils import run_bass_kernel_spmd

B, T, D = 4, 1024, 512
KC = 64            # 32 freqs x (cos,sin), flattened innermost dims of out
DSH = D // 2       # d-slice per core
NKC = DSH * KC     # free elements per t per core (16384)
BLK = 128          # rows per t-block
NBLK = T // BLK    # 8

BF16 = mybir.dt.bfloat16
F32 = mybir.dt.float32

# eviction split per block: DVE gets the tail groups (its queue first
# drains the next block's 8.6us TT); 3 and 4 alternate so DVE/ACT load
# balances across block pairs. Last block: DVE free (no next TT), so
# interleave odd/even for concurrency.
_DVE_GROUPS_BY_BLK = [
    (13, 14, 15), (12, 13, 14, 15), (13, 14, 15), (12, 13, 14, 15),
    (13, 14, 15), (12, 13, 14, 15), (13, 14, 15),
    (1, 3, 5, 7, 9, 11, 13, 15),
]
# stores: 4 chunks of 4 groups, emitted after the eviction of the
# chunk's last group; alternate sync/scalar queues. Last block: finer.
_CHUNK_AFTER = {3: (0, 4), 7: (4, 4), 11: (8, 4), 15: (12, 4)}
_CHUNK_AFTER_LAST = {
    2: (0, 3), 5: (3, 3), 7: (6, 2), 9: (8, 2),
    11: (10, 2), 13: (12, 2), 15: (14, 2),
}

LAST_RESULTS = None  # set by kernel(); test.py reads exec_time_ns from here


def _build_utri(pos_norm: np.ndarray) -> np.ndarray:
    """Stationary operands for all blocks, packed (128, NBLK*128) bf16."""
    pos = np.asarray(pos_norm).astype(np.float32)
    utri = np.zeros((128, NBLK * 128), np.float32)
    s = np.arange(128)[:, None]
    t = np.arange(128)[None, :]
    for k in range(NBLK):
        t0 = k * BLK
        utri[:, 128 * k : 128 * (k + 1)] = (s <= t) * pos[t0 : t0 + 128][None, :]
    return utri.astype(ml_dtypes.bfloat16)


def _build_program() -> bass.Bass:
    nc = bacc.Bacc("TRN2", target_bir_lowering=False, debug=False)
    x_d = nc.dram_tensor("x_shard", [T, DSH], BF16, kind="ExternalInput").ap()
    tw_d = nc.dram_tensor("tw", [T, KC], BF16, kind="ExternalInput").ap()
    utri_d = nc.dram_tensor("utri", [128, NBLK * 128], BF16, kind="ExternalInput").ap()
    twrep_d = nc.dram_tensor("twrep", [T, KC * 16], BF16, kind="ExternalInput").ap()
    out_d = nc.dram_tensor("out_shard", [T, NKC], BF16, kind="ExternalOutput").ap()

    with tile.TileContext(nc) as tc:
        with (
            tc.tile_pool(name="singles", bufs=1) as singles,
            tc.tile_pool(name="cp", bufs=3) as cp,
            tc.tile_pool(name="outp", bufs=2) as outp,
            tc.tile_pool(name="carryp", bufs=7) as carryp,
            tc.tile_pool(name="pmain", bufs=4, space="PSUM") as pmain,
        ):
            # loads, all 128-partition. sync queue: x block 0 (gates TT_0),
            # tw (gates the delta chain), rest of x. scalar queue: rep
            # block 0 + utri (gate TT_0 / block-0 matmuls), rest of rep.
            x_all = singles.tile([128, NBLK * DSH], BF16)
            x_all_v = x_all.rearrange("p (j d) -> p j d", j=NBLK)
            x_d_v = x_d.rearrange("(j p) d -> p j d", p=128)
            nc.sync.dma_start(out=x_all_v[:, 0:1], in_=x_d_v[:, 0:1])
            tw_all = singles.tile([128, NBLK * KC], BF16)
            nc.sync.dma_start(
                out=tw_all.rearrange("p (j k) -> p j k", j=NBLK),
                in_=tw_d.rearrange("(j p) k -> p j k", p=128),
            )
            nc.sync.dma_start(out=x_all_v[:, 1:NBLK], in_=x_d_v[:, 1:NBLK])
            rep_all = singles.tile([128, NBLK * KC * 16], BF16)
            rep_v0 = rep_all.rearrange("p (j r) -> p j r", j=NBLK)
            twrep_v = twrep_d.rearrange("(j p) r -> p j r", p=128)
            nc.scalar.dma_start(out=rep_v0[:, 0:1], in_=twrep_v[:, 0:1])
            utri_sb = singles.tile([128, NBLK * 128], BF16)
            nc.scalar.dma_start(out=utri_sb[:, :], in_=utri_d[:, :])
            nc.scalar.dma_start(out=rep_v0[:, 1:3], in_=twrep_v[:, 1:3])
            nc.scalar.dma_start(out=rep_v0[:, 3:NBLK], in_=twrep_v[:, 3:NBLK])

            def build_c(k, carry, nchunks=1):
                # contributions, kc-major: c[s, kc*DSH + d] = x[s,d] * tw[s,kc]
                # as bf16 tensor_tensor(s) in the DVE 2x mode; see v1 notes.
                rep16 = rep_all[:, k * KC * 16 : (k + 1) * KC * 16]
                x_sb = x_all[:, k * DSH : (k + 1) * DSH]
                c_sb = cp.tile([128, NKC], BF16)
                x_v3 = x_sb.rearrange("p (b c) -> p b c", c=16).unsqueeze(1)
                rep_v3 = rep16.rearrange("p (a c) -> p a c", c=16).unsqueeze(2)
                kcn = KC // nchunks
                for ci in range(nchunks):
                    ka, kb = ci * kcn, (ci + 1) * kcn
                    c_v = c_sb[:, ka * DSH : kb * DSH].rearrange(
                        "p (a b c) -> p a b c", b=16, c=16
                    )
                    nc.vector.tensor_mul(
                        c_v,
                        x_v3.broadcast_to((128, kcn, 16, 16)),
                        rep_v3[:, ka:kb].broadcast_to((128, kcn, 16, 16)),
                    )
                # fold the (precomputed) carry into c's first row: SWDGE DMA
                # with inline CCE add; utri row 0 is pos[t] for all t, so the
                # matmul finishes the block including the carry.
                if carry is not None:
                    nc.gpsimd.dma_start(
                        out=c_sb[0:1, :], in_=carry[:, :],
                        accum_op=mybir.AluOpType.add,
                    )
                return c_sb

            # TT_0 first (gates block 0; chunked so block-0 matmuls start
            # after the first quarter)
            c_cur = build_c(0, None, nchunks=4)

            # whole carry chain up front: carries[k] = sum_{j<k} tw_j^T @ x_j.
            # PE is idle before block 0's matmuls; the DVE adds slot in after
            # TT_0. Keeps carry_7 off the tail's critical path.
            carries = [None]
            carry_prev = None
            for k in range(NBLK - 1):
                delta = pmain.tile([KC, DSH], F32, tag="pg", name=f"delta{k}")
                nc.tensor.matmul(
                    delta[:, :],
                    lhsT=tw_all[:, k * KC : (k + 1) * KC],
                    rhs=x_all[:, k * DSH : (k + 1) * DSH],
                    start=True, stop=True,
                )
                carry_new = carryp.tile([KC, DSH], BF16, name=f"carry{k}")
                if carry_prev is None:
                    nc.vector.tensor_copy(carry_new[:, :], delta[:, :])
                else:
                    nc.vector.tensor_add(
                        carry_new[:, :], carry_prev[:, :], delta[:, :]
                    )
                carry_prev = carry_new
                carries.append(carry_new)

            for k in range(NBLK):
                if k + 1 < NBLK:
                    # next block's contributions build while this block runs
                    c_next = build_c(k + 1, carries[k + 1])
                else:
                    c_next = None

                og = outp.tile([128, NKC], BF16)
                dve_groups = _DVE_GROUPS_BY_BLK[k]
                chunk_after = _CHUNK_AFTER_LAST if k == NBLK - 1 else _CHUNK_AFTER
                lhsT = utri_sb[:, 128 * k : 128 * (k + 1)]
                n_ship = 0
                for gi in range(16):
                    pg = pmain.tile([128, 1024], F32, tag="pg", name=f"pg{k}_{gi}")
                    for jj in range(2):
                        nc.tensor.matmul(
                            pg[:, jj * 512 : (jj + 1) * 512],
                            lhsT=lhsT,
                            rhs=c_cur[:, (gi * 2 + jj) * 512 : (gi * 2 + jj + 1) * 512],
                            start=True, stop=True,
                        )
                    col = gi * 1024
                    if gi in dve_groups:
                        nc.vector.tensor_copy(og[:, col : col + 1024], pg[:, :])
                    else:
                        nc.scalar.copy(og[:, col : col + 1024], pg[:, :])
                    if gi in chunk_after:
                        c0, cg = chunk_after[gi]
                        n_ship += 1
                        eng = nc.sync if n_ship % 2 == 1 else nc.scalar
                        eng.dma_start(
                            out=out_d[k * BLK : (k + 1) * BLK,
                                      c0 * 1024 : (c0 + cg) * 1024],
                            in_=og[:, c0 * 1024 : (c0 + cg) * 1024],
                        )
                c_cur = c_next
    nc.compile()
    return nc


def kernel(**inputs) -> np.ndarray:
    global LAST_RESULTS
    x = np.asarray(inputs["x"])                       # (4,1024,512) bf16
    tw = np.asarray(inputs["twiddles"])               # (1024,32,2) bf16
    pos = np.asarray(inputs["pos_norm"])              # (1024,) bf16

    tw2 = np.ascontiguousarray(tw.reshape(T, KC))
    twrep = np.ascontiguousarray(np.repeat(tw2, 16, axis=1))
    utri = _build_utri(pos)

    in_maps = []
    for core in range(8):
        b, dh = core // 2, core % 2
        xs = np.ascontiguousarray(x[b, :, dh * DSH : (dh + 1) * DSH])
        in_maps.append(
            {"x_shard": xs, "tw": tw2, "utri": utri, "twrep": twrep}
        )

    nc = _build_program()
    res = run_bass_kernel_spmd(nc, in_maps, core_ids=list(range(8)))
    LAST_RESULTS = res

    out = np.empty((B, T, D, KC // 2, 2), dtype=x.dtype)
    for core in range(8):
        b, dh = core // 2, core % 2
        o = np.asarray(res.results[core]["out_shard"])  # (T, NKC) kc-major
        o = o.reshape(T, KC, DSH).transpose(0, 2, 1)    # -> (T, DSH, KC)
        out[b, :, dh * DSH : (dh + 1) * DSH, :, :] = o.reshape(T, DSH, KC // 2, 2)
    return out


if __name__ == "__main__":
    rng = np.random.default_rng(0)
    demo = {
        "x": rng.standard_normal((B, T, D), np.float32).astype(ml_dtypes.bfloat16),
        "twiddles": rng.standard_normal((T, KC // 2, 2), np.float32).astype(
            ml_dtypes.bfloat16
        ),
        "pos_norm": (1.0 / np.sqrt(np.arange(1, T + 1, dtype=np.float32))).astype(
            ml_dtypes.bfloat16
        ),
    }
    print(kernel(**demo).shape)


# revision 12
# speedup vs baseline: 1.2210x; 1.0014x over previous
"""Cumulative-FFT Trainium2 kernel (v3).

out[b,t,d,k,c] = pos_norm[t] * cumsum_t( x[b,t,d] * twiddles[t,k,c] )

Shapes (hardcoded): x (4,1024,512) bf16, twiddles (1024,32,2) bf16,
pos_norm (1024,) bf16  ->  out (4,1024,512,32,2) bf16.

Sharding: 8 cores = batch(4) x d_model-half(2). Each core computes a
(1024, 256*64) bf16 shard (32 MiB) -- data-parallel over B, tensor-parallel
over D, nothing crosses cores.

Per-core algorithm (v1 lineage): cumsum along t as per-block triangular
matmuls on the PE; the moving operand c holds the bf16 contributions
c[s, kc*256+d] = x[s,d]*tw[s,kc] (one 2x-mode DVE tensor_tensor against a
16x-replicated tw tile, 8.6us/block); the carry (column sums of previous
blocks) is folded into c's row 0 by an accumulating SWDGE DMA, so
utri[s,t] = pos[t0+t]*(s<=t) finishes each block in one matmul pass.

v3 changes over the 145.9us v1:
 - The whole carry chain (7 delta matmuls tw_k^T @ x_k + DVE adds) runs
   up front, right after the loads: carries stop gating late blocks
   (v1 lost ~5us waiting for carry_7 after TT_7).
 - Eviction split alternates 3/4 DVE groups per block (v1 fixed 4),
   balancing DVE (TT 8.6 + casts) against ACT across block pairs:
   2-block totals DVE 25.1us / ACT 25.1us -> ~12.6us/block steady
   (v1: 13.46).
 - Stores alternate between the qSync and qScalar HW-DGE queues (4
   chunks of 4 groups per block), halving per-queue load.
 - Head loads reordered: x block 0 + rep block 0 + utri first, so TT_0
   starts ~3us earlier.

A failed v2 for the record: moving the tw multiply into the PE
stationary (A_kc = utri*tw, built on DVE at half the TT cost) requires
re-adding the carry via K=1 rank-1 matmuls; ANY partial-K matmul (K=1
or K=32, packed or not) permanently throttles the PE clock to 1.2 GHz
(HAM never un-throttles; measured 75us of gapless back-to-back MMs all
at the cold rate), and full-K carry matmuls cost their column count
(+6.8us/block). Hard constraint: keep every matmul K=128.

Hard-won trace facts (v1, still binding):
 - HWDGE stripes a DMA across 16 SDMA engines only when the partition
   count divides by 16; all bulk DMAs here are 128-partition.
 - DVE TENSOR_TENSOR bf16 is capped at 2x mode ((58+FD/2)/0.96GHz);
   PSUM-source evictions are capped at 1x on both DVE ((120+FD)/0.96)
   and ACT ((172+FD)/1.2).
 - PSUM is 8 banks: pmain bufs=4 x 2 banks; the delta matmuls share the
   rotation via tag so PSUM never exceeds 8 banks.
 - Store floor: 32 MiB @ ~358 GB/s HBM-per-core = 11.7us/block.
"""

import sys

sys.path.insert(0, "/opt/trn_rl_repo")

import ml_dtypes
import numpy as np

import concourse.bass as bass
import concourse.mybir as mybir
import concourse.tile as tile
from concourse import bacc
import concourse.bass_utils as _bu
from concourse.bass_utils import run_bass_kernel_spmd

B, T, D = 4, 1024, 512
KC = 64            # 32 freqs x (cos,sin), flattened innermost dims of out
DSH = D // 2       # d-slice per core
NKC = DSH * KC     # free elements per t per core (16384)
BLK = 128          # rows per t-block
NBLK = T // BLK    # 8

BF16 = mybir.dt.bfloat16
F32 = mybir.dt.float32

# eviction split per block: DVE gets the tail groups (its queue first
# drains the next block's 8.6us TT); 3 and 4 alternate so DVE/ACT load
# balances across block pairs. Last block: DVE free (no next TT), so
# interleave odd/even for concurrency.
_DVE_GROUPS_BY_BLK = [
    (9, 11, 13, 15), (9, 11, 13, 15), (9, 11, 13, 15), (9, 11, 13, 15),
    (9, 11, 13, 15), (9, 11, 13, 15), (9, 11, 13, 15),
    (1, 3, 5, 7, 9, 11, 13, 15),
]
# stores: 4 chunks of 4 groups, emitted after the eviction of the
# chunk's last group; alternate sync/scalar queues. Last block: finer.
_CHUNK_AFTER = {3: (0, 4), 7: (4, 4), 11: (8, 4), 15: (12, 4)}
_CHUNK_AFTER_LAST = {
    2: (0, 3), 5: (3, 3), 7: (6, 2), 9: (8, 2),
    11: (10, 2), 13: (12, 2), 15: (14, 2),
}

LAST_RESULTS = None  # set by kernel(); test.py reads exec_time_ns from here


def _build_utri(pos_norm: np.ndarray) -> np.ndarray:
    """Stationary operands for all blocks, packed (128, NBLK*128) bf16."""
    pos = np.asarray(pos_norm).astype(np.float32)
    utri = np.zeros((128, NBLK * 128), np.float32)
    s = np.arange(128)[:, None]
    t = np.arange(128)[None, :]
    for k in range(NBLK):
        t0 = k * BLK
        utri[:, 128 * k : 128 * (k + 1)] = (s <= t) * pos[t0 : t0 + 128][None, :]
    return utri.astype(ml_dtypes.bfloat16)


def _build_program() -> bass.Bass:
    nc = bacc.Bacc("TRN2", target_bir_lowering=False, debug=False)
    x_d = nc.dram_tensor("x_shard", [T, DSH], BF16, kind="ExternalInput").ap()
    tw_d = nc.dram_tensor("tw", [T, KC], BF16, kind="ExternalInput").ap()
    utri_d = nc.dram_tensor("utri", [128, NBLK * 128], BF16, kind="ExternalInput").ap()
    twrep_d = nc.dram_tensor("twrep", [T, KC * 16], BF16, kind="ExternalInput").ap()
    out_d = nc.dram_tensor("out_shard", [T, NKC], BF16, kind="ExternalOutput").ap()

    with tile.TileContext(nc) as tc:
        with (
            tc.tile_pool(name="singles", bufs=1) as singles,
            tc.tile_pool(name="cp", bufs=3) as cp,
            tc.tile_pool(name="outp", bufs=2) as outp,
            tc.tile_pool(name="carryp", bufs=7) as carryp,
            tc.tile_pool(name="pmain", bufs=4, space="PSUM") as pmain,
        ):
            # loads, all 128-partition. sync queue: x block 0 (gates TT_0),
            # tw (gates the delta chain), rest of x. scalar queue: rep
            # block 0 + utri (gate TT_0 / block-0 matmuls), rest of rep.
            x_all = singles.tile([128, NBLK * DSH], BF16)
            x_all_v = x_all.rearrange("p (j d) -> p j d", j=NBLK)
            x_d_v = x_d.rearrange("(j p) d -> p j d", p=128)
            nc.sync.dma_start(out=x_all_v[:, 0:1], in_=x_d_v[:, 0:1])
            tw_all = singles.tile([128, NBLK * KC], BF16)
            nc.sync.dma_start(
                out=tw_all.rearrange("p (j k) -> p j k", j=NBLK),
                in_=tw_d.rearrange("(j p) k -> p j k", p=128),
            )
            nc.sync.dma_start(out=x_all_v[:, 1:NBLK], in_=x_d_v[:, 1:NBLK])
            rep_all = singles.tile([128, NBLK * KC * 16], BF16)
            rep_v0 = rep_all.rearrange("p (j r) -> p j r", j=NBLK)
            twrep_v = twrep_d.rearrange("(j p) r -> p j r", p=128)
            nc.scalar.dma_start(out=rep_v0[:, 0:1], in_=twrep_v[:, 0:1])
            utri_sb = singles.tile([128, NBLK * 128], BF16)
            nc.scalar.dma_start(out=utri_sb[:, :], in_=utri_d[:, :])
            nc.scalar.dma_start(out=rep_v0[:, 1:3], in_=twrep_v[:, 1:3])
            nc.scalar.dma_start(out=rep_v0[:, 3:NBLK], in_=twrep_v[:, 3:NBLK])

            def build_c(k, carry, nchunks=1):
                # contributions, kc-major: c[s, kc*DSH + d] = x[s,d] * tw[s,kc]
                # as bf16 tensor_tensor(s) in the DVE 2x mode; see v1 notes.
                rep16 = rep_all[:, k * KC * 16 : (k + 1) * KC * 16]
                x_sb = x_all[:, k * DSH : (k + 1) * DSH]
                c_sb = cp.tile([128, NKC], BF16)
                x_v3 = x_sb.rearrange("p (b c) -> p b c", c=16).unsqueeze(1)
                rep_v3 = rep16.rearrange("p (a c) -> p a c", c=16).unsqueeze(2)
                kcn = KC // nchunks
                for ci in range(nchunks):
                    ka, kb = ci * kcn, (ci + 1) * kcn
                    c_v = c_sb[:, ka * DSH : kb * DSH].rearrange(
                        "p (a b c) -> p a b c", b=16, c=16
                    )
                    nc.vector.tensor_mul(
                        c_v,
                        x_v3.broadcast_to((128, kcn, 16, 16)),
                        rep_v3[:, ka:kb].broadcast_to((128, kcn, 16, 16)),
                    )
                # fold the (precomputed) carry into c's first row: SWDGE DMA
                # with inline CCE add; utri row 0 is pos[t] for all t, so the
                # matmul finishes the block including the carry.
                if carry is not None:
                    nc.gpsimd.dma_start(
                        out=c_sb[0:1, :], in_=carry[:, :],
                        accum_op=mybir.AluOpType.add,
                    )
                return c_sb

            # TT_0 first (gates block 0; chunked so block-0 matmuls start
            # after the first quarter)
            c_cur = build_c(0, None, nchunks=4)

            # whole carry chain up front: carries[k] = sum_{j<k} tw_j^T @ x_j.
            # PE is idle before block 0's matmuls; the DVE adds slot in after
            # TT_0. Keeps carry_7 off the tail's critical path.
            carries = [None]
            carry_prev = None
            for k in range(NBLK - 1):
                delta = pmain.tile([KC, DSH], F32, tag="pg", name=f"delta{k}")
                nc.tensor.matmul(
                    delta[:, :],
                    lhsT=tw_all[:, k * KC : (k + 1) * KC],
                    rhs=x_all[:, k * DSH : (k + 1) * DSH],
                    start=True, stop=True,
                )
                carry_new = carryp.tile([KC, DSH], BF16, name=f"carry{k}")
                if carry_prev is None:
                    nc.vector.tensor_copy(carry_new[:, :], delta[:, :])
                else:
                    nc.vector.tensor_add(
                        carry_new[:, :], carry_prev[:, :], delta[:, :]
                    )
                carry_prev = carry_new
                carries.append(carry_new)

            for k in range(NBLK):
                if k + 1 < NBLK:
                    # next block's contributions build while this block runs
                    c_next = build_c(k + 1, carries[k + 1])
                else:
                    c_next = None

                og = outp.tile([128, NKC], BF16)
                dve_groups = _DVE_GROUPS_BY_BLK[k]
                chunk_after = _CHUNK_AFTER_LAST if k == NBLK - 1 else _CHUNK_AFTER
                lhsT = utri_sb[:, 128 * k : 128 * (k + 1)]
                n_ship = 0
                for gi in range(16):
                    pg = pmain.tile([128, 1024], F32, tag="pg", name=f"pg{k}_{gi}")
                    for jj in range(2):
                        nc.tensor.matmul(
                            pg[:, jj * 512 : (jj + 1) * 512],
                            lhsT=lhsT,
                            rhs=c_cur[:, (gi * 2 + jj) * 512 : (gi * 2 + jj + 1) * 512],
                            start=True, stop=True,
                        )
                    col = gi * 1024
                    if gi in dve_groups:
                        nc.vector.tensor_copy(og[:, col : col + 1024], pg[:, :])
                    else:
                        nc.scalar.copy(og[:, col : col + 1024], pg[:, :])
                    if gi in chunk_after:
                        c0, cg = chunk_after[gi]
                        n_ship += 1
                        eng = nc.sync if n_ship % 2 == 1 else nc.scalar
                        eng.dma_start(
                            out=out_d[k * BLK : (k + 1) * BLK,
                                      c0 * 1024 : (c0 + cg) * 1024],
                            in_=og[:, c0 * 1024 : (c0 + cg) * 1024],
                        )
                c_cur = c_next
    nc.compile()
    return nc


def kernel(**inputs) -> np.ndarray:
    global LAST_RESULTS
    x = np.asarray(inputs["x"])                       # (4,1024,512) bf16
    tw = np.asarray(inputs["twiddles"])               # (1024,32,2) bf16
    pos = np.asarray(inputs["pos_norm"])              # (1024,) bf16

    tw2 = np.ascontiguousarray(tw.reshape(T, KC))
    twrep = np.ascontiguousarray(np.repeat(tw2, 16, axis=1))
    utri = _build_utri(pos)

    in_maps = []
    for core in range(8):
        b, dh = core // 2, core % 2
        xs = np.ascontiguousarray(x[b, :, dh * DSH : (dh + 1) * DSH])
        in_maps.append(
            {"x_shard": xs, "tw": tw2, "utri": utri, "twrep": twrep}
        )

    nc = _build_program()
    res = run_bass_kernel_spmd(nc, in_maps, core_ids=list(range(8)))
    LAST_RESULTS = res

    out = np.empty((B, T, D, KC // 2, 2), dtype=x.dtype)
    for core in range(8):
        b, dh = core // 2, core % 2
        o = np.asarray(res.results[core]["out_shard"])  # (T, NKC) kc-major
        o = o.reshape(T, KC, DSH).transpose(0, 2, 1)    # -> (T, DSH, KC)
        out[b, :, dh * DSH : (dh + 1) * DSH, :, :] = o.reshape(T, DSH, KC // 2, 2)
    return out


if __name__ == "__main__":
    rng = np.random.default_rng(0)
    demo = {
        "x": rng.standard_normal((B, T, D), np.float32).astype(ml_dtypes.bfloat16),
        "twiddles": rng.standard_normal((T, KC // 2, 2), np.float32).astype(
            ml_dtypes.bfloat16
        ),
        "pos_norm": (1.0 / np.sqrt(np.arange(1, T + 1, dtype=np.float32))).astype(
            ml_dtypes.bfloat16
        ),
    }
    print(kernel(**demo).shape)


# revision 14
# speedup vs baseline: 1.3493x; 1.1051x over previous
"""Cumulative-FFT Trainium2 kernel (v3).

out[b,t,d,k,c] = pos_norm[t] * cumsum_t( x[b,t,d] * twiddles[t,k,c] )

Shapes (hardcoded): x (4,1024,512) bf16, twiddles (1024,32,2) bf16,
pos_norm (1024,) bf16  ->  out (4,1024,512,32,2) bf16.

Sharding: 8 cores = batch(4) x d_model-half(2). Each core computes a
(1024, 256*64) bf16 shard (32 MiB) -- data-parallel over B, tensor-parallel
over D, nothing crosses cores.

Per-core algorithm (v1 lineage): cumsum along t as per-block triangular
matmuls on the PE; the moving operand c holds the bf16 contributions
c[s, kc*256+d] = x[s,d]*tw[s,kc] (one 2x-mode DVE tensor_tensor against a
16x-replicated tw tile, 8.6us/block); the carry (column sums of previous
blocks) is folded into c's row 0 by an accumulating SWDGE DMA, so
utri[s,t] = pos[t0+t]*(s<=t) finishes each block in one matmul pass.

v3 changes over the 145.9us v1:
 - The whole carry chain (7 delta matmuls tw_k^T @ x_k + DVE adds) runs
   up front, right after the loads: carries stop gating late blocks
   (v1 lost ~5us waiting for carry_7 after TT_7).
 - Eviction split alternates 3/4 DVE groups per block (v1 fixed 4),
   balancing DVE (TT 8.6 + casts) against ACT across block pairs:
   2-block totals DVE 25.1us / ACT 25.1us -> ~12.6us/block steady
   (v1: 13.46).
 - Stores alternate between the qSync and qScalar HW-DGE queues (4
   chunks of 4 groups per block), halving per-queue load.
 - Head loads reordered: x block 0 + rep block 0 + utri first, so TT_0
   starts ~3us earlier.

A failed v2 for the record: moving the tw multiply into the PE
stationary (A_kc = utri*tw, built on DVE at half the TT cost) requires
re-adding the carry via K=1 rank-1 matmuls; ANY partial-K matmul (K=1
or K=32, packed or not) permanently throttles the PE clock to 1.2 GHz
(HAM never un-throttles; measured 75us of gapless back-to-back MMs all
at the cold rate), and full-K carry matmuls cost their column count
(+6.8us/block). Hard constraint: keep every matmul K=128.

Hard-won trace facts (v1, still binding):
 - HWDGE stripes a DMA across 16 SDMA engines only when the partition
   count divides by 16; all bulk DMAs here are 128-partition.
 - DVE TENSOR_TENSOR bf16 is capped at 2x mode ((58+FD/2)/0.96GHz);
   PSUM-source evictions are capped at 1x on both DVE ((120+FD)/0.96)
   and ACT ((172+FD)/1.2).
 - PSUM is 8 banks: pmain bufs=4 x 2 banks; the delta matmuls share the
   rotation via tag so PSUM never exceeds 8 banks.
 - Store floor: 32 MiB @ ~358 GB/s HBM-per-core = 11.7us/block.
"""

import sys

sys.path.insert(0, "/opt/trn_rl_repo")

import ml_dtypes
import numpy as np

import concourse.bass as bass
import concourse.mybir as mybir
import concourse.tile as tile
from concourse import bacc
import concourse.bass_utils as _bu
from concourse.bass_utils import run_bass_kernel_spmd

B, T, D = 4, 1024, 512
KC = 64            # 32 freqs x (cos,sin), flattened innermost dims of out
DSH = D // 2       # d-slice per core
NKC = DSH * KC     # free elements per t per core (16384)
BLK = 128          # rows per t-block
NBLK = T // BLK    # 8

BF16 = mybir.dt.bfloat16
F32 = mybir.dt.float32

# eviction split per block: DVE gets the tail groups (its queue first
# drains the next block's 8.6us TT); 3 and 4 alternate so DVE/ACT load
# balances across block pairs. Last block: DVE free (no next TT), so
# interleave odd/even for concurrency.
_DVE_GROUPS_BY_BLK = [
    (9, 10, 11, 12), (9, 10, 11, 12), (9, 10, 11, 12), (9, 10, 11, 12),
    (9, 10, 11, 12), (9, 10, 11, 12), (9, 10, 11, 12),
    (1, 3, 5, 7, 9, 11, 13, 15),
]
# stores: 4 chunks of 4 groups, emitted after the eviction of the
# chunk's last group; alternate sync/scalar queues. Last block: finer.
_CHUNK_AFTER = {3: (0, 4), 7: (4, 4), 11: (8, 4), 15: (12, 4)}
_CHUNK_AFTER_LAST = {
    2: (0, 3), 5: (3, 3), 7: (6, 2), 9: (8, 2),
    11: (10, 2), 13: (12, 2), 15: (14, 2),
}

LAST_RESULTS = None  # set by kernel(); test.py reads exec_time_ns from here


def _build_utri(pos_norm: np.ndarray) -> np.ndarray:
    """Stationary operands for all blocks, packed (128, NBLK*128) bf16."""
    pos = np.asarray(pos_norm).astype(np.float32)
    utri = np.zeros((128, NBLK * 128), np.float32)
    s = np.arange(128)[:, None]
    t = np.arange(128)[None, :]
    for k in range(NBLK):
        t0 = k * BLK
        utri[:, 128 * k : 128 * (k + 1)] = (s <= t) * pos[t0 : t0 + 128][None, :]
    return utri.astype(ml_dtypes.bfloat16)


def _build_program() -> bass.Bass:
    nc = bacc.Bacc("TRN2", target_bir_lowering=False, debug=False)
    x_d = nc.dram_tensor("x_shard", [T, DSH], BF16, kind="ExternalInput").ap()
    tw_d = nc.dram_tensor("tw", [T, KC], BF16, kind="ExternalInput").ap()
    utri_d = nc.dram_tensor("utri", [128, NBLK * 128], BF16, kind="ExternalInput").ap()
    twrep_d = nc.dram_tensor("twrep", [T, KC * 16], BF16, kind="ExternalInput").ap()
    out_d = nc.dram_tensor("out_shard", [T, NKC], BF16, kind="ExternalOutput").ap()

    with tile.TileContext(nc) as tc:
        with (
            tc.tile_pool(name="singles", bufs=1) as singles,
            tc.tile_pool(name="cp", bufs=3) as cp,
            tc.tile_pool(name="outp", bufs=2) as outp,
            tc.tile_pool(name="carryp", bufs=7) as carryp,
            tc.tile_pool(name="pmain", bufs=4, space="PSUM") as pmain,
        ):
            # loads, all 128-partition. sync queue: x block 0 (gates TT_0),
            # tw (gates the delta chain), rest of x. scalar queue: rep
            # block 0 + utri (gate TT_0 / block-0 matmuls), rest of rep.
            x_all = singles.tile([128, NBLK * DSH], BF16)
            x_all_v = x_all.rearrange("p (j d) -> p j d", j=NBLK)
            x_d_v = x_d.rearrange("(j p) d -> p j d", p=128)
            nc.sync.dma_start(out=x_all_v[:, 0:1], in_=x_d_v[:, 0:1])
            tw_all = singles.tile([128, NBLK * KC], BF16)
            nc.sync.dma_start(
                out=tw_all.rearrange("p (j k) -> p j k", j=NBLK),
                in_=tw_d.rearrange("(j p) k -> p j k", p=128),
            )
            nc.sync.dma_start(out=x_all_v[:, 1:NBLK], in_=x_d_v[:, 1:NBLK])
            rep_all = singles.tile([128, NBLK * KC * 16], BF16)
            rep_v0 = rep_all.rearrange("p (j r) -> p j r", j=NBLK)
            twrep_v = twrep_d.rearrange("(j p) r -> p j r", p=128)
            nc.scalar.dma_start(out=rep_v0[:, 0:1], in_=twrep_v[:, 0:1])
            utri_sb = singles.tile([128, NBLK * 128], BF16)
            nc.scalar.dma_start(out=utri_sb[:, :], in_=utri_d[:, :])
            nc.scalar.dma_start(out=rep_v0[:, 1:3], in_=twrep_v[:, 1:3])
            nc.scalar.dma_start(out=rep_v0[:, 3:NBLK], in_=twrep_v[:, 3:NBLK])

            def build_c(k, carry, nchunks=1):
                # contributions, kc-major: c[s, kc*DSH + d] = x[s,d] * tw[s,kc]
                # as bf16 tensor_tensor(s) in the DVE 2x mode; see v1 notes.
                rep16 = rep_all[:, k * KC * 16 : (k + 1) * KC * 16]
                x_sb = x_all[:, k * DSH : (k + 1) * DSH]
                c_sb = cp.tile([128, NKC], BF16)
                x_v3 = x_sb.rearrange("p (b c) -> p b c", c=16).unsqueeze(1)
                rep_v3 = rep16.rearrange("p (a c) -> p a c", c=16).unsqueeze(2)
                kcn = KC // nchunks
                for ci in range(nchunks):
                    ka, kb = ci * kcn, (ci + 1) * kcn
                    c_v = c_sb[:, ka * DSH : kb * DSH].rearrange(
                        "p (a b c) -> p a b c", b=16, c=16
                    )
                    nc.vector.tensor_mul(
                        c_v,
                        x_v3.broadcast_to((128, kcn, 16, 16)),
                        rep_v3[:, ka:kb].broadcast_to((128, kcn, 16, 16)),
                    )
                # fold the (precomputed) carry into c's first row: SWDGE DMA
                # with inline CCE add; utri row 0 is pos[t] for all t, so the
                # matmul finishes the block including the carry.
                if carry is not None:
                    nc.gpsimd.dma_start(
                        out=c_sb[0:1, :], in_=carry[:, :],
                        accum_op=mybir.AluOpType.add,
                    )
                return c_sb

            # TT_0 first (gates block 0; chunked so block-0 matmuls start
            # after the first quarter)
            c_cur = build_c(0, None, nchunks=4)

            # whole carry chain up front: carries[k] = sum_{j<k} tw_j^T @ x_j.
            # PE is idle before block 0's matmuls; the DVE adds slot in after
            # TT_0. Keeps carry_7 off the tail's critical path.
            carries = [None]
            carry_prev = None
            for k in range(NBLK - 1):
                delta = pmain.tile([KC, DSH], F32, tag="pg", name=f"delta{k}")
                nc.tensor.matmul(
                    delta[:, :],
                    lhsT=tw_all[:, k * KC : (k + 1) * KC],
                    rhs=x_all[:, k * DSH : (k + 1) * DSH],
                    start=True, stop=True,
                )
                carry_new = carryp.tile([KC, DSH], BF16, name=f"carry{k}")
                if carry_prev is None:
                    nc.vector.tensor_copy(carry_new[:, :], delta[:, :])
                else:
                    nc.vector.tensor_add(
                        carry_new[:, :], carry_prev[:, :], delta[:, :]
                    )
                carry_prev = carry_new
                carries.append(carry_new)

            for k in range(NBLK):
                if k + 1 < NBLK:
                    # next block's contributions build while this block runs
                    c_next = build_c(k + 1, carries[k + 1])
                else:
                    c_next = None

                og = outp.tile([128, NKC], BF16)
                dve_groups = _DVE_GROUPS_BY_BLK[k]
                chunk_after = _CHUNK_AFTER_LAST if k == NBLK - 1 else _CHUNK_AFTER
                lhsT = utri_sb[:, 128 * k : 128 * (k + 1)]
                n_ship = 0
                for gi in range(16):
                    pg = pmain.tile([128, 1024], F32, tag="pg", name=f"pg{k}_{gi}")
                    for jj in range(2):
                        nc.tensor.matmul(
                            pg[:, jj * 512 : (jj + 1) * 512],
                            lhsT=lhsT,
                            rhs=c_cur[:, (gi * 2 + jj) * 512 : (gi * 2 + jj + 1) * 512],
                            start=True, stop=True,
                        )
                    col = gi * 1024
                    if gi in dve_groups:
                        nc.vector.tensor_copy(og[:, col : col + 1024], pg[:, :])
                    else:
                        nc.scalar.copy(og[:, col : col + 1024], pg[:, :])
                    if gi in chunk_after:
                        c0, cg = chunk_after[gi]
                        n_ship += 1
                        # steady-state stores go ONLY on the dedicated Sync
                        # queue: a dma_start in the ACT instruction stream
                        # blocks head-of-line on DVE's tail casts and stalls
                        # the next block's copies. Scalar queue helps only in
                        # the last block (no ACT work after).
                        eng = (
                            nc.scalar
                            if k == NBLK - 1 and n_ship > 4
                            else nc.sync
                        )
                        eng.dma_start(
                            out=out_d[k * BLK : (k + 1) * BLK,
                                      c0 * 1024 : (c0 + cg) * 1024],
                            in_=og[:, c0 * 1024 : (c0 + cg) * 1024],
                        )
                c_cur = c_next
    nc.compile()
    return nc


def kernel(**inputs) -> np.ndarray:
    global LAST_RESULTS
    x = np.asarray(inputs["x"])                       # (4,1024,512) bf16
    tw = np.asarray(inputs["twiddles"])               # (1024,32,2) bf16
    pos = np.asarray(inputs["pos_norm"])              # (1024,) bf16

    tw2 = np.ascontiguousarray(tw.reshape(T, KC))
    twrep = np.ascontiguousarray(np.repeat(tw2, 16, axis=1))
    utri = _build_utri(pos)

    in_maps = []
    for core in range(8):
        b, dh = core // 2, core % 2
        xs = np.ascontiguousarray(x[b, :, dh * DSH : (dh + 1) * DSH])
        in_maps.append(
            {"x_shard": xs, "tw": tw2, "utri": utri, "twrep": twrep}
        )

    nc = _build_program()
    res = run_bass_kernel_spmd(nc, in_maps, core_ids=list(range(8)))
    LAST_RESULTS = res

    out = np.empty((B, T, D, KC // 2, 2), dtype=x.dtype)
    for core in range(8):
        b, dh = core // 2, core % 2
        o = np.asarray(res.results[core]["out_shard"])  # (T, NKC) kc-major
        o = o.reshape(T, KC, DSH).transpose(0, 2, 1)    # -> (T, DSH, KC)
        out[b, :, dh * DSH : (dh + 1) * DSH, :, :] = o.reshape(T, DSH, KC // 2, 2)
    return out


if __name__ == "__main__":
    rng = np.random.default_rng(0)
    demo = {
        "x": rng.standard_normal((B, T, D), np.float32).astype(ml_dtypes.bfloat16),
        "twiddles": rng.standard_normal((T, KC // 2, 2), np.float32).astype(
            ml_dtypes.bfloat16
        ),
        "pos_norm": (1.0 / np.sqrt(np.arange(1, T + 1, dtype=np.float32))).astype(
            ml_dtypes.bfloat16
        ),
    }
    print(kernel(**demo).shape)


# revision 16
# speedup vs baseline: 1.3793x; 1.0222x over previous
"""Cumulative-FFT Trainium2 kernel (v3).

out[b,t,d,k,c] = pos_norm[t] * cumsum_t( x[b,t,d] * twiddles[t,k,c] )

Shapes (hardcoded): x (4,1024,512) bf16, twiddles (1024,32,2) bf16,
pos_norm (1024,) bf16  ->  out (4,1024,512,32,2) bf16.

Sharding: 8 cores = batch(4) x d_model-half(2). Each core computes a
(1024, 256*64) bf16 shard (32 MiB) -- data-parallel over B, tensor-parallel
over D, nothing crosses cores.

Per-core algorithm (v1 lineage): cumsum along t as per-block triangular
matmuls on the PE; the moving operand c holds the bf16 contributions
c[s, kc*256+d] = x[s,d]*tw[s,kc] (one 2x-mode DVE tensor_tensor against a
16x-replicated tw tile, 8.6us/block); the carry (column sums of previous
blocks) is folded into c's row 0 by an accumulating SWDGE DMA, so
utri[s,t] = pos[t0+t]*(s<=t) finishes each block in one matmul pass.

v3 changes over the 145.9us v1:
 - The whole carry chain (7 delta matmuls tw_k^T @ x_k + DVE adds) runs
   up front, right after the loads: carries stop gating late blocks
   (v1 lost ~5us waiting for carry_7 after TT_7).
 - Eviction split alternates 3/4 DVE groups per block (v1 fixed 4),
   balancing DVE (TT 8.6 + casts) against ACT across block pairs:
   2-block totals DVE 25.1us / ACT 25.1us -> ~12.6us/block steady
   (v1: 13.46).
 - Stores alternate between the qSync and qScalar HW-DGE queues (4
   chunks of 4 groups per block), halving per-queue load.
 - Head loads reordered: x block 0 + rep block 0 + utri first, so TT_0
   starts ~3us earlier.

A failed v2 for the record: moving the tw multiply into the PE
stationary (A_kc = utri*tw, built on DVE at half the TT cost) requires
re-adding the carry via K=1 rank-1 matmuls; ANY partial-K matmul (K=1
or K=32, packed or not) permanently throttles the PE clock to 1.2 GHz
(HAM never un-throttles; measured 75us of gapless back-to-back MMs all
at the cold rate), and full-K carry matmuls cost their column count
(+6.8us/block). Hard constraint: keep every matmul K=128.

Hard-won trace facts (v1, still binding):
 - HWDGE stripes a DMA across 16 SDMA engines only when the partition
   count divides by 16; all bulk DMAs here are 128-partition.
 - DVE TENSOR_TENSOR bf16 is capped at 2x mode ((58+FD/2)/0.96GHz);
   PSUM-source evictions are capped at 1x on both DVE ((120+FD)/0.96)
   and ACT ((172+FD)/1.2).
 - PSUM is 8 banks: pmain bufs=4 x 2 banks; the delta matmuls share the
   rotation via tag so PSUM never exceeds 8 banks.
 - Store floor: 32 MiB @ ~358 GB/s HBM-per-core = 11.7us/block.
"""

import sys

sys.path.insert(0, "/opt/trn_rl_repo")

import ml_dtypes
import numpy as np

import concourse.bass as bass
import concourse.mybir as mybir
import concourse.tile as tile
from concourse import bacc
import concourse.bass_utils as _bu
from concourse.bass_utils import run_bass_kernel_spmd

B, T, D = 4, 1024, 512
KC = 64            # 32 freqs x (cos,sin), flattened innermost dims of out
DSH = D // 2       # d-slice per core
NKC = DSH * KC     # free elements per t per core (16384)
BLK = 128          # rows per t-block
NBLK = T // BLK    # 8

BF16 = mybir.dt.bfloat16
F32 = mybir.dt.float32

# eviction split per block: DVE gets the tail groups (its queue first
# drains the next block's 8.6us TT); 3 and 4 alternate so DVE/ACT load
# balances across block pairs. Last block: DVE free (no next TT), so
# interleave odd/even for concurrency.
_DVE_GROUPS_BY_BLK = [
    (9, 10, 11, 12), (9, 10, 11, 12), (9, 10, 11, 12), (9, 10, 11, 12),
    (9, 10, 11, 12), (9, 10, 11, 12), (9, 10, 11, 12),
    (1, 3, 5, 7, 9, 11, 13, 15),
]
# stores: 4 chunks of 4 groups, emitted after the eviction of the
# chunk's last group; alternate sync/scalar queues. Last block: finer.
_CHUNK_AFTER = {3: (0, 4), 7: (4, 4), 11: (8, 4), 15: (12, 4)}
_CHUNK_AFTER_LAST = {
    2: (0, 3), 5: (3, 3), 7: (6, 2), 9: (8, 2),
    11: (10, 2), 13: (12, 2), 15: (14, 2),
}

LAST_RESULTS = None  # set by kernel(); test.py reads exec_time_ns from here


def _build_utri(pos_norm: np.ndarray) -> np.ndarray:
    """Stationary operands for all blocks, packed (128, NBLK*128) bf16."""
    pos = np.asarray(pos_norm).astype(np.float32)
    utri = np.zeros((128, NBLK * 128), np.float32)
    s = np.arange(128)[:, None]
    t = np.arange(128)[None, :]
    for k in range(NBLK):
        t0 = k * BLK
        utri[:, 128 * k : 128 * (k + 1)] = (s <= t) * pos[t0 : t0 + 128][None, :]
    return utri.astype(ml_dtypes.bfloat16)


def _build_program() -> bass.Bass:
    nc = bacc.Bacc("TRN2", target_bir_lowering=False, debug=False)
    x_d = nc.dram_tensor("x_shard", [T, DSH], BF16, kind="ExternalInput").ap()
    tw_d = nc.dram_tensor("tw", [T, KC], BF16, kind="ExternalInput").ap()
    utri_d = nc.dram_tensor("utri", [128, NBLK * 128], BF16, kind="ExternalInput").ap()
    twrep_d = nc.dram_tensor("twrep", [T, KC * 16], BF16, kind="ExternalInput").ap()
    out_d = nc.dram_tensor("out_shard", [T, NKC], BF16, kind="ExternalOutput").ap()

    with tile.TileContext(nc) as tc:
        with (
            tc.tile_pool(name="singles", bufs=1) as singles,
            tc.tile_pool(name="cp", bufs=3) as cp,
            tc.tile_pool(name="outp", bufs=2) as outp,
            tc.tile_pool(name="pmain", bufs=4, space="PSUM") as pmain,
        ):
            # loads, all 128-partition. sync queue: x block 0 (gates TT_0),
            # tw (gates the delta chain), rest of x. scalar queue: rep
            # block 0 + utri (gate TT_0 / block-0 matmuls), rest of rep.
            x_all = singles.tile([128, NBLK * DSH], BF16)
            x_all_v = x_all.rearrange("p (j d) -> p j d", j=NBLK)
            x_d_v = x_d.rearrange("(j p) d -> p j d", p=128)
            nc.sync.dma_start(out=x_all_v[:, 0:1], in_=x_d_v[:, 0:1])
            tw_all = singles.tile([128, NBLK * KC], BF16)
            nc.sync.dma_start(
                out=tw_all.rearrange("p (j k) -> p j k", j=NBLK),
                in_=tw_d.rearrange("(j p) k -> p j k", p=128),
            )
            nc.sync.dma_start(out=x_all_v[:, 1:NBLK], in_=x_d_v[:, 1:NBLK])
            rep_all = singles.tile([128, NBLK * KC * 16], BF16)
            rep_v0 = rep_all.rearrange("p (j r) -> p j r", j=NBLK)
            twrep_v = twrep_d.rearrange("(j p) r -> p j r", p=128)
            nc.scalar.dma_start(out=rep_v0[:, 0:1], in_=twrep_v[:, 0:1])
            utri_sb = singles.tile([128, NBLK * 128], BF16)
            nc.scalar.dma_start(out=utri_sb[:, :], in_=utri_d[:, :])
            nc.scalar.dma_start(out=rep_v0[:, 1:3], in_=twrep_v[:, 1:3])
            nc.scalar.dma_start(out=rep_v0[:, 3:NBLK], in_=twrep_v[:, 3:NBLK])

            def build_c(k, carry, nchunks=1):
                # contributions, kc-major: c[s, kc*DSH + d] = x[s,d] * tw[s,kc]
                # as bf16 tensor_tensor(s) in the DVE 2x mode; see v1 notes.
                rep16 = rep_all[:, k * KC * 16 : (k + 1) * KC * 16]
                x_sb = x_all[:, k * DSH : (k + 1) * DSH]
                c_sb = cp.tile([128, NKC], BF16)
                x_v3 = x_sb.rearrange("p (b c) -> p b c", c=16).unsqueeze(1)
                rep_v3 = rep16.rearrange("p (a c) -> p a c", c=16).unsqueeze(2)
                kcn = KC // nchunks
                for ci in range(nchunks):
                    ka, kb = ci * kcn, (ci + 1) * kcn
                    c_v = c_sb[:, ka * DSH : kb * DSH].rearrange(
                        "p (a b c) -> p a b c", b=16, c=16
                    )
                    nc.vector.tensor_mul(
                        c_v,
                        x_v3.broadcast_to((128, kcn, 16, 16)),
                        rep_v3[:, ka:kb].broadcast_to((128, kcn, 16, 16)),
                    )
                # fold the (precomputed) carry into c's first row: SWDGE DMA
                # with inline CCE add; utri row 0 is pos[t] for all t, so the
                # matmul finishes the block including the carry.
                if carry is not None:
                    nc.gpsimd.dma_start(
                        out=c_sb[0:1, :], in_=carry[:, :],
                        accum_op=mybir.AluOpType.add,
                    )
                return c_sb

            # TT_0 first (gates block 0; chunked so block-0 matmuls start
            # after the first quarter)
            c_cur = build_c(0, None, nchunks=4)

            # whole carry chain up front: carries[k] = sum_{j<k} tw_j^T @ x_j.
            # PE is idle before block 0's matmuls. The chain's PSUM consumers
            # must NOT sit on DVE (its queue is busy with TT_0/TT_1 and the
            # pg-slot rotation would stall the PE FIFO on delta_4): ACT
            # evicts each delta to SBUF, GpSimd (also idle early) runs the
            # prefix adds. Keeps carry_7 off the tail's critical path too.
            dsb_all = singles.tile([KC, (NBLK - 1) * DSH], BF16)
            carr_all = singles.tile([KC, (NBLK - 2) * DSH], BF16)
            carries = [None]
            for k in range(NBLK - 1):
                delta = pmain.tile([KC, DSH], F32, tag="pg", name=f"delta{k}")
                nc.tensor.matmul(
                    delta[:, :],
                    lhsT=tw_all[:, k * KC : (k + 1) * KC],
                    rhs=x_all[:, k * DSH : (k + 1) * DSH],
                    start=True, stop=True,
                )
                dsb = dsb_all[:, k * DSH : (k + 1) * DSH]
                nc.scalar.copy(dsb, delta[:, :])
                if k == 0:
                    carries.append(dsb)
                else:
                    carry_new = carr_all[:, (k - 1) * DSH : k * DSH]
                    nc.gpsimd.tensor_add(carry_new, carries[k], dsb)
                    carries.append(carry_new)

            for k in range(NBLK):
                if k + 1 < NBLK:
                    # next block's contributions build while this block runs
                    c_next = build_c(k + 1, carries[k + 1])
                else:
                    c_next = None

                og = outp.tile([128, NKC], BF16)
                dve_groups = _DVE_GROUPS_BY_BLK[k]
                chunk_after = _CHUNK_AFTER_LAST if k == NBLK - 1 else _CHUNK_AFTER
                lhsT = utri_sb[:, 128 * k : 128 * (k + 1)]
                n_ship = 0
                for gi in range(16):
                    pg = pmain.tile([128, 1024], F32, tag="pg", name=f"pg{k}_{gi}")
                    for jj in range(2):
                        nc.tensor.matmul(
                            pg[:, jj * 512 : (jj + 1) * 512],
                            lhsT=lhsT,
                            rhs=c_cur[:, (gi * 2 + jj) * 512 : (gi * 2 + jj + 1) * 512],
                            start=True, stop=True,
                        )
                    col = gi * 1024
                    if gi in dve_groups:
                        nc.vector.tensor_copy(og[:, col : col + 1024], pg[:, :])
                    else:
                        nc.scalar.copy(og[:, col : col + 1024], pg[:, :])
                    if gi in chunk_after:
                        c0, cg = chunk_after[gi]
                        n_ship += 1
                        # steady-state stores go ONLY on the dedicated Sync
                        # queue: a dma_start in the ACT instruction stream
                        # blocks head-of-line on DVE's tail casts and stalls
                        # the next block's copies. Scalar queue helps only in
                        # the last block (no ACT work after).
                        eng = (
                            nc.scalar
                            if k == NBLK - 1 and n_ship > 4
                            else nc.sync
                        )
                        eng.dma_start(
                            out=out_d[k * BLK : (k + 1) * BLK,
                                      c0 * 1024 : (c0 + cg) * 1024],
                            in_=og[:, c0 * 1024 : (c0 + cg) * 1024],
                        )
                c_cur = c_next
    nc.compile()
    return nc


def kernel(**inputs) -> np.ndarray:
    global LAST_RESULTS
    x = np.asarray(inputs["x"])                       # (4,1024,512) bf16
    tw = np.asarray(inputs["twiddles"])               # (1024,32,2) bf16
    pos = np.asarray(inputs["pos_norm"])              # (1024,) bf16

    tw2 = np.ascontiguousarray(tw.reshape(T, KC))
    twrep = np.ascontiguousarray(np.repeat(tw2, 16, axis=1))
    utri = _build_utri(pos)

    in_maps = []
    for core in range(8):
        b, dh = core // 2, core % 2
        xs = np.ascontiguousarray(x[b, :, dh * DSH : (dh + 1) * DSH])
        in_maps.append(
            {"x_shard": xs, "tw": tw2, "utri": utri, "twrep": twrep}
        )

    nc = _build_program()
    res = run_bass_kernel_spmd(nc, in_maps, core_ids=list(range(8)))
    LAST_RESULTS = res

    out = np.empty((B, T, D, KC // 2, 2), dtype=x.dtype)
    for core in range(8):
        b, dh = core // 2, core % 2
        o = np.asarray(res.results[core]["out_shard"])  # (T, NKC) kc-major
        o = o.reshape(T, KC, DSH).transpose(0, 2, 1)    # -> (T, DSH, KC)
        out[b, :, dh * DSH : (dh + 1) * DSH, :, :] = o.reshape(T, DSH, KC // 2, 2)
    return out


if __name__ == "__main__":
    rng = np.random.default_rng(0)
    demo = {
        "x": rng.standard_normal((B, T, D), np.float32).astype(ml_dtypes.bfloat16),
        "twiddles": rng.standard_normal((T, KC // 2, 2), np.float32).astype(
            ml_dtypes.bfloat16
        ),
        "pos_norm": (1.0 / np.sqrt(np.arange(1, T + 1, dtype=np.float32))).astype(
            ml_dtypes.bfloat16
        ),
    }
    print(kernel(**demo).shape)
